# revision 19
# baseline (speedup 1.0000x reference)
"""Expert-parallel MoE kernel for one TRN2 chip (8 NeuronCores) — v2.

All routing/bookkeeping is host-side; the device runs a pure FFN plus a
destination-grouped AllToAll:

  - Host computes gating in f64 (verified to reproduce the f32 reference
    top-2 exactly: min 2nd/3rd-logit margin is 2e-5, ~10x above f32 matmul
    rounding), builds per-core gather/scatter lists, normalizes + transposes
    + bf16-casts the weights, and blocks them for contiguous h-major DMA.
  - Core e owns expert e. Device: stream w1/w2 blocks in, gather compacted
    tokens (transposed) per 512-token group, GEMM1 (+Silu, bias folded),
    GEMM2 (+bias), scatter-add bf16 outputs into a destination-grouped
    AllToAll buffer, fire chunked AllToAlls that overlap later groups.
  - Receive: each core gathers the two expert contributions for each of its
    1024 tokens from the A2A output, adds them in fp32, DMAs the result out.

Compared to v1 this removes the on-device gating -> A2A -> prefix-sum ->
one-hot list build critical-path head, the 17MB ReduceScatter (A2A wire is
~5MB vs ~17MB), the rs_in zeroing, and the final cast pass.
"""

import numpy as np

import concourse.bass as bass
import concourse.mybir as mybir
import concourse.tile as tile
from concourse import bacc
from concourse.library_config import mlp

F32 = mybir.dt.float32
BF16 = mybir.dt.bfloat16
I16 = mybir.dt.int16

AX = mybir.AxisListType
OP = mybir.AluOpType
ACT = mybir.ActivationFunctionType

T, D, H, E = 8192, 1024, 4096, 8
NCORES = 8
TSL = T // NCORES
ND = D // 128
NH = H // 128


class Plan:
    """Host-computed routing plan; device program is keyed on `key()`."""

    def __init__(self, C, NC, PADC, use_b1, use_b2, gsize=512,
                 no_recv=False, no_coll=False, no_scatter=False, no_ffn=False):
        self.no_recv = no_recv
        self.no_coll = no_coll
        self.no_scatter = no_scatter
        self.no_ffn = no_ffn
        self.C = C                    # per-expert compacted capacity
        self.NC = NC                  # number of A2A chunks
        self.PADC = PADC              # slots per (expert,dst) pair per chunk
        self.use_b1 = use_b1
        self.use_b2 = use_b2
        self.groups = []
        off = 0
        while off < C:
            s = min(gsize, C - off)
            self.groups.append((off, s))
            off += s
        assert all(s % 128 == 0 for _, s in self.groups)
        self.R = NC * E * PADC        # a2a buffer rows
        assert self.R % 128 == 0
        # A2A chunk c may only fire once every position < B_c has been
        # scattered. P_{c+1} = sum_d min((c+1)*PADC, cnt_ed) <= 8*PADC*(c+1)
        # for every core, so firing after the first group whose end covers
        # that bound is uniform across cores (program must be SPMD).
        self.fire_after = {gi: [] for gi in range(len(self.groups))}
        for c in range(NC):
            bound = min(C, E * PADC * (c + 1))
            for gi, (goff, s) in enumerate(self.groups):
                if goff + s >= bound:
                    self.fire_after[gi].append(c)
                    break

    def key(self):
        return (self.C, self.NC, self.PADC, self.use_b1, self.use_b2,
                tuple(self.groups), self.no_recv, self.no_coll,
                self.no_scatter, self.no_ffn)


def build_moe(nc, plan: Plan):
    C, NC, PADC, R = plan.C, plan.NC, plan.PADC, plan.R
    B8 = E * PADC                     # rows per a2a chunk
    NLI = C // 16 + 2 * (TSL // 16)

    # ---------------- kernel I/O ----------------
    xt = nc.dram_tensor("xt", [128, C * ND], BF16,
                        kind="ExternalInput").ap()
    w1d = nc.dram_tensor("w1d", [NH, 128, ND * 128], BF16,
                         kind="ExternalInput").ap()
    w2d = nc.dram_tensor("w2d", [NH, 128, D], BF16, kind="ExternalInput").ap()
    lists = nc.dram_tensor("lists", [128, NLI], I16, kind="ExternalInput").ap()
    if plan.use_b1:
        b1w = nc.dram_tensor("b1w", [128, NH], F32, kind="ExternalInput").ap()
    if plan.use_b2:
        b2r = nc.dram_tensor("b2r", [1, D], BF16, kind="ExternalInput").ap()
        ones1b = nc.dram_tensor("ones1b", [1, 128], BF16,
                                kind="ExternalInput").ap()
    out_ext = nc.dram_tensor("out", [TSL, D], F32, kind="ExternalOutput").ap()

    # ---------------- internal DRAM ----------------
    # +128 dump rows: unfilled slots scatter there (negative scatter
    # indices wedge the device); the A2A only moves rows [0, R).
    a2a_in = nc.dram_tensor("a2a_in", [R + 128, D], BF16).ap()
    a2a_out = nc.dram_tensor("a2a_out", [R, D], BF16).ap()

    rg = [list(range(NCORES))]

    sct_off = 0
    rcva_off = C // 16
    rcvb_off = rcva_off + TSL // 16

    with tile.TileContext(nc) as tc:
        nc.gpsimd.load_library(mlp)

        with (
            tc.tile_pool(name="consts", bufs=1) as cpool,
            tc.tile_pool(name="wbig", bufs=1) as wpool,
        ):
            li = cpool.tile([128, NLI], I16)
            nc.sync.dma_start(li[:], lists)
            if plan.use_b1:
                b1w_sb = cpool.tile([128, NH], F32)
                nc.sync.dma_start(b1w_sb[:], b1w)
            if plan.use_b2:
                b2r_sb = cpool.tile([1, D], BF16)
                nc.sync.dma_start(b2r_sb[:], b2r)
                ones1b_sb = cpool.tile([1, 128], BF16)
                nc.sync.dma_start(ones1b_sb[:], ones1b)

            w1T = wpool.tile([128, NH, ND, 128], BF16)  # [d_in, hc, dc, h_in]
            w2T = wpool.tile([128, NH, D], BF16)        # [h_in, hc, d]
            for hc in range(NH):
                nc.scalar.dma_start(
                    w1T[:, hc, :, :].rearrange("p a b -> p (a b)"), w1d[hc])
            for hc in range(NH):
                nc.scalar.dma_start(w2T[:, hc, :], w2d[hc])

            # zero the a2a input buffer (scatter-add accumulates into it);
            # separate queue so weight DMAs keep priority on scalar's queue
            zt = cpool.tile([128, D], BF16)
            nc.gpsimd.memset(zt[:], 0.0)
            for j in range(R // 128 + 1):
                nc.scalar.dma_start(a2a_in[j * 128:(j + 1) * 128, :], zt[:])

            # ============ expert FFN over compacted tokens ============
            with (
                tc.tile_pool(name="gemm", bufs=2) as gpool,
                tc.tile_pool(name="psG", bufs=3, space="PSUM") as psp,
            ):
                for gi, (goff, S) in enumerate(plan.groups):
                    xgt = gpool.tile([128, ND, S], BF16, tag="xgt", bufs=2)
                    nc.sync.dma_start(
                        xgt[:].rearrange("p a b -> p (a b)"),
                        xt[:, ND * goff:ND * (goff + S)])
                    og = gpool.tile([128, S // 128, D], BF16, tag="og", bufs=2)
                    if plan.no_ffn:
                        nc.gpsimd.memset(og[:], 0.0)
                    if not plan.no_ffn:
                        hT = gpool.tile([128, NH, S], BF16, tag="hT", bufs=1)
                        for hc in range(NH):
                            ph = psp.tile([128, S], F32, tag="ph", bufs=3)
                            for dc in range(ND):
                                nc.tensor.matmul(
                                    ph[:], lhsT=w1T[:, hc, dc, :],
                                    rhs=xgt[:, dc, :],
                                    start=(dc == 0), stop=(dc == ND - 1))
                            if plan.use_b1:
                                nc.scalar.activation(hT[:, hc, :], ph[:],
                                                     ACT.Silu,
                                                     bias=b1w_sb[:, hc:hc + 1])
                            else:
                                nc.scalar.activation(hT[:, hc, :], ph[:],
                                                     ACT.Silu)
                        for tb in range(S // 128):
                            for dn in range(D // 512):
                                po = psp.tile([128, 512], F32, tag="po", bufs=3)
                                for hc in range(NH):
                                    nc.tensor.matmul(
                                        po[:],
                                        lhsT=hT[:, hc, tb * 128:(tb + 1) * 128],
                                        rhs=w2T[:, hc, dn * 512:(dn + 1) * 512],
                                        start=(hc == 0),
                                        stop=(hc == NH - 1 and not plan.use_b2))
                                if plan.use_b2:
                                    nc.tensor.matmul(
                                        po[:], lhsT=ones1b_sb[:],
                                        rhs=b2r_sb[:, dn * 512:(dn + 1) * 512],
                                        start=False, stop=True)
                                nc.vector.tensor_copy(
                                    og[:, tb, dn * 512:(dn + 1) * 512], po[:])
                    if plan.no_scatter:
                        nc.gpsimd.dma_start(
                            a2a_in[goff:goff + S, :].rearrange(
                                "(a p) b -> p a b", p=128),
                            og[:])
                    else:
                        nc.gpsimd.dma_scatter_add(
                            a2a_in[goff:, :], og[:],
                            li[:, sct_off + goff // 16:
                               sct_off + (goff + S) // 16],
                            S, S, D)
                    if not plan.no_coll:
                        for c in plan.fire_after[gi]:
                            nc.gpsimd.collective_compute(
                                "AllToAll", OP.bypass, replica_groups=rg,
                                ins=[a2a_in[c * B8:(c + 1) * B8, :]],
                                outs=[a2a_out[c * B8:(c + 1) * B8, :]])

            # ============ receive: out[t] = contrib_A[t] + contrib_B[t] ====
            with tc.tile_pool(name="outp", bufs=1) as opool:
                if plan.no_recv:
                    for c in range(TSL // 128):
                        rb = opool.tile([128, D], BF16, tag="rb", bufs=2)
                        nc.sync.dma_start(rb[:],
                                          a2a_out[c * 128:(c + 1) * 128, :])
                        rs = opool.tile([128, D], F32, tag="rs", bufs=4)
                        nc.vector.tensor_copy(rs[:], rb[:])
                        nc.sync.dma_start(out_ext[c * 128:(c + 1) * 128, :],
                                          rs[:])
                else:
                    ga = opool.tile([128, TSL // 128, D], BF16, tag="ga")
                    gb = opool.tile([128, TSL // 128, D], BF16, tag="gb")
                    nc.gpsimd.dma_gather(
                        ga[:], a2a_out, li[:, rcva_off:rcva_off + TSL // 16],
                        TSL, TSL, D, transpose=False)
                    nc.gpsimd.dma_gather(
                        gb[:], a2a_out, li[:, rcvb_off:rcvb_off + TSL // 16],
                        TSL, TSL, D, transpose=False)
                    for c in range(TSL // 128):
                        rs = opool.tile([128, D], F32, tag="rs", bufs=4)
                        nc.vector.tensor_tensor(rs[:], ga[:, c, :],
                                                gb[:, c, :], op=OP.add)
                        nc.sync.dma_start(out_ext[c * 128:(c + 1) * 128, :],
                                          rs[:])

    return nc


# ================= host-side planning =================

def _routing(x, gate_v, gate_g, gate_b):
    """Exact (f64) top-2 routing; matches the f32 jax reference."""
    xf = np.asarray(x, np.float64).reshape(T, D)
    gv = np.asarray(gate_v, np.float64)
    n = np.maximum(np.sqrt((gv * gv).sum(-1, keepdims=True)), 1e-12)
    gw = gv / n * np.asarray(gate_g, np.float64).reshape(E, 1)
    logits = xf @ gw.T + np.asarray(gate_b, np.float64).reshape(1, E)
    part = np.argpartition(-logits, 2, axis=1)[:, :2]
    return part  # [T, 2] unordered top-2 expert ids


def make_plan(x, gate_v, gate_g, gate_b, b1, b2, NC=4):
    top2 = _routing(x, gate_v, gate_g, gate_b)
    # tokens_ed[e][d]: sorted token ids of slice d routed to expert e
    tokens_ed = [[None] * NCORES for _ in range(E)]
    onehot = np.zeros((T, E), bool)
    onehot[np.arange(T), top2[:, 0]] = True
    onehot[np.arange(T), top2[:, 1]] = True
    for e in range(E):
        toks = np.nonzero(onehot[:, e])[0]
        for d in range(NCORES):
            tokens_ed[e][d] = toks[(toks >= d * TSL) & (toks < (d + 1) * TSL)]
    cnt_ed = np.array([[len(tokens_ed[e][d]) for d in range(NCORES)]
                       for e in range(E)])
    cnt_e = cnt_ed.sum(1)
    C = int(-(-cnt_e.max() // 128) * 128)
    max_ed = int(cnt_ed.max())
    PADC = int(-(-max_ed // NC // 16) * 16)
    use_b1 = bool(np.any(np.asarray(b1) != 0))
    use_b2 = bool(np.any(np.asarray(b2) != 0))
    plan = Plan(C, NC, PADC, use_b1, use_b2)
    plan.tokens_ed = tokens_ed
    plan.cnt_ed = cnt_ed
    plan.top2 = top2
    return plan


def _wrap16(v):
    """[n] int array -> [128, n/16] int16 list layout (entry i at
    [i % 16, i // 16], replicated across the 8 gpsimd channels)."""
    v = np.asarray(v)
    assert v.size % 16 == 0
    w = v.reshape(-1, 16).T.astype(np.int16)  # [16, n/16]
    return np.ascontiguousarray(np.tile(w, (8, 1)))


def make_in_maps(plan: Plan, x, gate_v, gate_g, gate_b, w1_v, w1_g, b1,
                 w2_v, w2_g, b2):
    import ml_dtypes

    bf = ml_dtypes.bfloat16
    C, NC, PADC = plan.C, plan.NC, plan.PADC
    f32 = np.float32

    xf = np.asarray(x, f32).reshape(T, D)
    xbf = xf.astype(bf)

    w1_v = np.asarray(w1_v, f32)
    w1_g = np.asarray(w1_g, f32)
    w2_v = np.asarray(w2_v, f32)
    w2_g = np.asarray(w2_g, f32)
    b1 = np.asarray(b1, f32)
    b2 = np.asarray(b2, f32)

    in_maps = []
    for e in range(E):
        # ---- normalized weights, blocked for h-major DMA ----
        n1 = np.maximum(np.sqrt((w1_v[e] ** 2).sum(-1, keepdims=True)), 1e-12)
        W1n = (w1_v[e] / n1 * w1_g[e][:, None]).astype(bf)        # [H, D]
        n2 = np.maximum(np.sqrt((w2_v[e] ** 2).sum(-1, keepdims=True)), 1e-12)
        W2n = (w2_v[e] / n2 * w2_g[e][:, None]).astype(bf)        # [D, H]
        w1blk = np.ascontiguousarray(
            W1n.reshape(NH, 128, ND, 128).transpose(0, 3, 2, 1)
            .reshape(NH, 128, ND * 128))                          # [hc,d_in,(dc h_in)]
        w2blk = np.ascontiguousarray(
            W2n.T.reshape(NH, 128, D))                            # [hc,h_in,d]

        # ---- position ordering: (chunk, dst, slot) nested ----
        xglist = np.full(C, -1, np.int64)
        sctlist = np.full(C, 0, np.int64)
        p = 0
        for c in range(NC):
            for d in range(NCORES):
                toks = plan.tokens_ed[e][d]
                j0, j1 = c * PADC, min((c + 1) * PADC, len(toks))
                for j in range(j0, j1):
                    xglist[p] = toks[j]
                    sctlist[p] = c * E * PADC + d * PADC + (j - j0)
                    p += 1
        assert p == int(plan.cnt_ed[e].sum())
        for q in range(p, C):           # trailing pad slots:
            sctlist[q] = plan.R + (q % 128)   # scatter into a2a dump rows
        # indices are relative to each group's sliced out AP (a2a_in[goff:]),
        # which keeps later scatters disjoint from in-flight A2A chunk reads
        for goff, S in plan.groups:
            sctlist[goff:goff + S] -= goff
        assert (sctlist >= 0).all()
        # precompacted transposed x: per group g, free block
        # [ND*goff, ND*(goff+S)) holds [dc, tok] with d_in on partitions
        xc = np.zeros((C, D), dtype=bf)
        xc[:p] = xbf[xglist[:p]]
        parts = []
        for goff, S in plan.groups:
            blk = xc[goff:goff + S].reshape(S, ND, 128).transpose(2, 1, 0)
            parts.append(blk.reshape(128, ND * S))
        xtb = np.ascontiguousarray(np.concatenate(parts, axis=1))

        # ---- receive lists for core e's own token slice ----
        rcva = np.zeros(TSL, np.int64)
        rcvb = np.zeros(TSL, np.int64)
        jrank = {}
        for src in range(E):
            toks = plan.tokens_ed[src][e]
            for j, t in enumerate(toks):
                row = (j // PADC) * E * PADC + src * PADC + (j % PADC)
                jrank[(src, t)] = row
        for i in range(TSL):
            t = e * TSL + i
            e1, e2 = sorted(plan.top2[t])
            rcva[i] = jrank[(e1, t)]
            rcvb[i] = jrank[(e2, t)]

        lists = np.concatenate([
            _wrap16(sctlist), _wrap16(rcva), _wrap16(rcvb)], axis=1)

        im = {
            "xt": xtb,
            "w1d": w1blk,
            "w2d": w2blk,
            "lists": lists,
        }
        if plan.use_b1:
            im["b1w"] = np.ascontiguousarray(b1[e].reshape(NH, 128).T)
        if plan.use_b2:
            im["b2r"] = b2[e].reshape(1, D).astype(bf)
            im["ones1b"] = np.ones((1, 128), dtype=bf)
        in_maps.append(im)
    return in_maps


_COMPILED = {}


def get_compiled(plan: Plan):
    key = plan.key()
    if key not in _COMPILED:
        nc = bacc.Bacc("TRN2", target_bir_lowering=False, debug=False,
                       num_devices=NCORES)
        build_moe(nc, plan)
        nc.compile()
        _COMPILED[key] = nc
    return _COMPILED[key]


def unpermute(plan: Plan, shards):
    """Core d's out shard is token slice d in natural order."""
    return np.asarray(shards, np.float32).reshape(T, D)


def kernel(x, gate_v, gate_g, gate_b, w1_v, w1_g, b1, w2_v, w2_g, b2):
    from concourse.bass_utils import run_bass_kernel_spmd

    plan = make_plan(x, gate_v, gate_g, gate_b, b1, b2)
    nc = get_compiled(plan)
    in_maps = make_in_maps(plan, x, gate_v, gate_g, gate_b, w1_v, w1_g, b1,
                           w2_v, w2_g, b2)
    res = run_bass_kernel_spmd(nc, in_maps, core_ids=list(range(NCORES)))
    shards = [res.results[i]["out"] for i in range(NCORES)]
    out = unpermute(plan, np.stack(shards, axis=0))
    B, S_, D_ = x.shape
    return out.reshape(B, S_, D_)


# revision 21
# speedup vs baseline: 1.4804x; 1.4804x over previous
"""Expert-parallel MoE kernel for one TRN2 chip (8 NeuronCores) — v2.

All routing/bookkeeping is host-side; the device runs a pure FFN plus a
destination-grouped AllToAll:

  - Host computes gating in f64 (verified to reproduce the f32 reference
    top-2 exactly: min 2nd/3rd-logit margin is 2e-5, ~10x above f32 matmul
    rounding), builds per-core gather/scatter lists, normalizes + transposes
    + bf16-casts the weights, and blocks them for contiguous h-major DMA.
  - Core e owns expert e. Device: stream w1/w2 blocks in, gather compacted
    tokens (transposed) per 512-token group, GEMM1 (+Silu, bias folded),
    GEMM2 (+bias), scatter-add bf16 outputs into a destination-grouped
    AllToAll buffer, fire chunked AllToAlls that overlap later groups.
  - Receive: each core gathers the two expert contributions for each of its
    1024 tokens from the A2A output, adds them in fp32, DMAs the result out.

Compared to v1 this removes the on-device gating -> A2A -> prefix-sum ->
one-hot list build critical-path head, the 17MB ReduceScatter (A2A wire is
~5MB vs ~17MB), the rs_in zeroing, and the final cast pass.
"""

import numpy as np

import concourse.bass as bass
import concourse.mybir as mybir
import concourse.tile as tile
from concourse import bacc
from concourse.library_config import mlp

F32 = mybir.dt.float32
BF16 = mybir.dt.bfloat16
I16 = mybir.dt.int16

AX = mybir.AxisListType
OP = mybir.AluOpType
ACT = mybir.ActivationFunctionType

T, D, H, E = 8192, 1024, 4096, 8
NCORES = 8
TSL = T // NCORES
ND = D // 128
NH = H // 128


class Plan:
    """Host-computed routing plan; device program is keyed on `key()`."""

    def __init__(self, C, NC, PADC, use_b1, use_b2, gsize=512,
                 no_recv=False, no_coll=False, no_scatter=False, no_ffn=False):
        self.no_recv = no_recv
        self.no_coll = no_coll
        self.no_scatter = no_scatter
        self.no_ffn = no_ffn
        self.C = C                    # per-expert compacted capacity
        self.NC = NC                  # number of A2A chunks
        self.PADC = PADC              # slots per (expert,dst) pair per chunk
        self.use_b1 = use_b1
        self.use_b2 = use_b2
        self.groups = []
        off = 0
        while off < C:
            s = min(gsize, C - off)
            self.groups.append((off, s))
            off += s
        assert all(s % 128 == 0 for _, s in self.groups)
        self.R = NC * E * PADC        # a2a buffer rows
        assert self.R % 128 == 0
        # A2A chunk c may only fire once every position < B_c has been
        # scattered. P_{c+1} = sum_d min((c+1)*PADC, cnt_ed) <= 8*PADC*(c+1)
        # for every core, so firing after the first group whose end covers
        # that bound is uniform across cores (program must be SPMD).
        self.fire_after = {gi: [] for gi in range(len(self.groups))}
        for c in range(NC):
            bound = min(C, E * PADC * (c + 1))
            for gi, (goff, s) in enumerate(self.groups):
                if goff + s >= bound:
                    self.fire_after[gi].append(c)
                    break

    def key(self):
        return (self.C, self.NC, self.PADC, self.use_b1, self.use_b2,
                tuple(self.groups), self.no_recv, self.no_coll,
                self.no_scatter, self.no_ffn)


def build_moe(nc, plan: Plan):
    C, NC, PADC, R = plan.C, plan.NC, plan.PADC, plan.R
    B8 = E * PADC                     # rows per a2a chunk
    NLI = C // 16 + 2 * (TSL // 16)

    # ---------------- kernel I/O ----------------
    xt = nc.dram_tensor("xt", [128, C * ND], BF16,
                        kind="ExternalInput").ap()
    w1d = nc.dram_tensor("w1d", [NH, 128, ND * 128], BF16,
                         kind="ExternalInput").ap()
    w2d = nc.dram_tensor("w2d", [NH, 128, D], BF16, kind="ExternalInput").ap()
    lists = nc.dram_tensor("lists", [128, NLI], I16, kind="ExternalInput").ap()
    if plan.use_b1:
        b1w = nc.dram_tensor("b1w", [128, NH], F32, kind="ExternalInput").ap()
    if plan.use_b2:
        b2r = nc.dram_tensor("b2r", [1, D], BF16, kind="ExternalInput").ap()
        ones1b = nc.dram_tensor("ones1b", [1, 128], BF16,
                                kind="ExternalInput").ap()
    out_ext = nc.dram_tensor("out", [TSL, D], F32, kind="ExternalOutput").ap()

    # ---------------- internal DRAM ----------------
    # +128 dump rows: unfilled slots scatter there (negative scatter
    # indices wedge the device); the A2A only moves rows [0, R).
    a2a_in = nc.dram_tensor("a2a_in", [R + 128, D], BF16).ap()
    a2a_out = nc.dram_tensor("a2a_out", [R, D], BF16).ap()

    rg = [list(range(NCORES))]

    sct_off = 0
    rcva_off = C // 16
    rcvb_off = rcva_off + TSL // 16

    with tile.TileContext(nc) as tc:
        nc.gpsimd.load_library(mlp)

        with (
            tc.tile_pool(name="consts", bufs=1) as cpool,
            tc.tile_pool(name="wbig", bufs=1) as wpool,
        ):
            li = cpool.tile([128, NLI], I16)
            nc.sync.dma_start(li[:], lists)
            if plan.use_b1:
                b1w_sb = cpool.tile([128, NH], F32)
                nc.sync.dma_start(b1w_sb[:], b1w)
            if plan.use_b2:
                b2r_sb = cpool.tile([1, D], BF16)
                nc.sync.dma_start(b2r_sb[:], b2r)
                ones1b_sb = cpool.tile([1, 128], BF16)
                nc.sync.dma_start(ones1b_sb[:], ones1b)

            w1T = wpool.tile([128, NH, ND, 128], BF16)  # [d_in, hc, dc, h_in]
            w2T = wpool.tile([128, NH, D], BF16)        # [h_in, hc, d]
            for hc in range(NH):
                nc.scalar.dma_start(
                    w1T[:, hc, :, :].rearrange("p a b -> p (a b)"), w1d[hc])
            for hc in range(NH):
                nc.scalar.dma_start(w2T[:, hc, :], w2d[hc])

            # zero the a2a input buffer (scatter-add accumulates into it);
            # separate queue so weight DMAs keep priority on scalar's queue
            zt = cpool.tile([128, D], BF16)
            nc.gpsimd.memset(zt[:], 0.0)
            for j in range(R // 128 + 1):
                nc.scalar.dma_start(a2a_in[j * 128:(j + 1) * 128, :], zt[:])

            # ============ expert FFN over compacted tokens ============
            with (
                tc.tile_pool(name="gemm", bufs=2) as gpool,
                tc.tile_pool(name="psG", bufs=3, space="PSUM") as psp,
            ):
                for gi, (goff, S) in enumerate(plan.groups):
                    xgt = gpool.tile([128, ND, S], BF16, tag="xgt", bufs=2)
                    nc.sync.dma_start(
                        xgt[:].rearrange("p a b -> p (a b)"),
                        xt[:, ND * goff:ND * (goff + S)])
                    og = gpool.tile([128, S // 128, D], BF16, tag="og", bufs=2)
                    if plan.no_ffn:
                        nc.gpsimd.memset(og[:], 0.0)
                    if not plan.no_ffn:
                        hT = gpool.tile([128, NH, S], BF16, tag="hT", bufs=1)
                        for hc in range(NH):
                            ph = psp.tile([128, S], F32, tag="ph", bufs=3)
                            for dc in range(ND):
                                nc.tensor.matmul(
                                    ph[:], lhsT=w1T[:, hc, dc, :],
                                    rhs=xgt[:, dc, :],
                                    start=(dc == 0), stop=(dc == ND - 1))
                            if plan.use_b1:
                                nc.scalar.activation(hT[:, hc, :], ph[:],
                                                     ACT.Silu,
                                                     bias=b1w_sb[:, hc:hc + 1])
                            else:
                                nc.scalar.activation(hT[:, hc, :], ph[:],
                                                     ACT.Silu)
                        for tb in range(S // 128):
                            for dn in range(D // 512):
                                po = psp.tile([128, 512], F32, tag="po", bufs=3)
                                for hc in range(NH):
                                    nc.tensor.matmul(
                                        po[:],
                                        lhsT=hT[:, hc, tb * 128:(tb + 1) * 128],
                                        rhs=w2T[:, hc, dn * 512:(dn + 1) * 512],
                                        start=(hc == 0),
                                        stop=(hc == NH - 1 and not plan.use_b2))
                                if plan.use_b2:
                                    nc.tensor.matmul(
                                        po[:], lhsT=ones1b_sb[:],
                                        rhs=b2r_sb[:, dn * 512:(dn + 1) * 512],
                                        start=False, stop=True)
                                nc.vector.tensor_copy(
                                    og[:, tb, dn * 512:(dn + 1) * 512], po[:])
                    if plan.no_scatter:
                        nc.gpsimd.dma_start(
                            a2a_in[goff:goff + S, :].rearrange(
                                "(a p) b -> p a b", p=128),
                            og[:])
                    else:
                        nc.gpsimd.dma_scatter_add(
                            a2a_in[goff:, :], og[:],
                            li[:, sct_off + goff // 16:
                               sct_off + (goff + S) // 16],
                            S, S, D)
                    if not plan.no_coll:
                        for c in plan.fire_after[gi]:
                            nc.gpsimd.collective_compute(
                                "AllToAll", OP.bypass, replica_groups=rg,
                                ins=[a2a_in[c * B8:(c + 1) * B8, :]],
                                outs=[a2a_out[c * B8:(c + 1) * B8, :]])

            # ============ receive: out[t] = contrib_A[t] + contrib_B[t] ====
            with tc.tile_pool(name="outp", bufs=1) as opool:
                if plan.no_recv:
                    for c in range(TSL // 128):
                        rb = opool.tile([128, D], BF16, tag="rb", bufs=2)
                        nc.sync.dma_start(rb[:],
                                          a2a_out[c * 128:(c + 1) * 128, :])
                        rs = opool.tile([128, D], F32, tag="rs", bufs=4)
                        nc.vector.tensor_copy(rs[:], rb[:])
                        nc.sync.dma_start(out_ext[c * 128:(c + 1) * 128, :],
                                          rs[:])
                else:
                    ga = opool.tile([128, TSL // 128, D], BF16, tag="ga")
                    gb = opool.tile([128, TSL // 128, D], BF16, tag="gb")
                    nc.gpsimd.dma_gather(
                        ga[:], a2a_out, li[:, rcva_off:rcva_off + TSL // 16],
                        TSL, TSL, D, transpose=False)
                    nc.gpsimd.dma_gather(
                        gb[:], a2a_out, li[:, rcvb_off:rcvb_off + TSL // 16],
                        TSL, TSL, D, transpose=False)
                    for c in range(TSL // 128):
                        rs = opool.tile([128, D], F32, tag="rs", bufs=4)
                        nc.vector.tensor_tensor(rs[:], ga[:, c, :],
                                                gb[:, c, :], op=OP.add)
                        nc.sync.dma_start(out_ext[c * 128:(c + 1) * 128, :],
                                          rs[:])

    return nc


# ================= host-side planning =================

def _routing(x, gate_v, gate_g, gate_b):
    """Exact (f64) top-2 routing; matches the f32 jax reference."""
    xf = np.asarray(x, np.float64).reshape(T, D)
    gv = np.asarray(gate_v, np.float64)
    n = np.maximum(np.sqrt((gv * gv).sum(-1, keepdims=True)), 1e-12)
    gw = gv / n * np.asarray(gate_g, np.float64).reshape(E, 1)
    logits = xf @ gw.T + np.asarray(gate_b, np.float64).reshape(1, E)
    part = np.argpartition(-logits, 2, axis=1)[:, :2]
    return part  # [T, 2] unordered top-2 expert ids


def make_plan(x, gate_v, gate_g, gate_b, b1, b2, NC=5):
    top2 = _routing(x, gate_v, gate_g, gate_b)
    # tokens_ed[e][d]: sorted token ids of slice d routed to expert e
    tokens_ed = [[None] * NCORES for _ in range(E)]
    onehot = np.zeros((T, E), bool)
    onehot[np.arange(T), top2[:, 0]] = True
    onehot[np.arange(T), top2[:, 1]] = True
    for e in range(E):
        toks = np.nonzero(onehot[:, e])[0]
        for d in range(NCORES):
            tokens_ed[e][d] = toks[(toks >= d * TSL) & (toks < (d + 1) * TSL)]
    cnt_ed = np.array([[len(tokens_ed[e][d]) for d in range(NCORES)]
                       for e in range(E)])
    cnt_e = cnt_ed.sum(1)
    C = int(-(-cnt_e.max() // 128) * 128)
    max_ed = int(cnt_ed.max())
    PADC = int(-(-max_ed // NC // 16) * 16)
    use_b1 = bool(np.any(np.asarray(b1) != 0))
    use_b2 = bool(np.any(np.asarray(b2) != 0))
    plan = Plan(C, NC, PADC, use_b1, use_b2)
    plan.tokens_ed = tokens_ed
    plan.cnt_ed = cnt_ed
    plan.top2 = top2
    return plan


def _wrap16(v):
    """[n] int array -> [128, n/16] int16 list layout (entry i at
    [i % 16, i // 16], replicated across the 8 gpsimd channels)."""
    v = np.asarray(v)
    assert v.size % 16 == 0
    w = v.reshape(-1, 16).T.astype(np.int16)  # [16, n/16]
    return np.ascontiguousarray(np.tile(w, (8, 1)))


def make_in_maps(plan: Plan, x, gate_v, gate_g, gate_b, w1_v, w1_g, b1,
                 w2_v, w2_g, b2):
    import ml_dtypes

    bf = ml_dtypes.bfloat16
    C, NC, PADC = plan.C, plan.NC, plan.PADC
    f32 = np.float32

    xf = np.asarray(x, f32).reshape(T, D)
    xbf = xf.astype(bf)

    w1_v = np.asarray(w1_v, f32)
    w1_g = np.asarray(w1_g, f32)
    w2_v = np.asarray(w2_v, f32)
    w2_g = np.asarray(w2_g, f32)
    b1 = np.asarray(b1, f32)
    b2 = np.asarray(b2, f32)

    in_maps = []
    for e in range(E):
        # ---- normalized weights, blocked for h-major DMA ----
        n1 = np.maximum(np.sqrt((w1_v[e] ** 2).sum(-1, keepdims=True)), 1e-12)
        W1n = (w1_v[e] / n1 * w1_g[e][:, None]).astype(bf)        # [H, D]
        n2 = np.maximum(np.sqrt((w2_v[e] ** 2).sum(-1, keepdims=True)), 1e-12)
        W2n = (w2_v[e] / n2 * w2_g[e][:, None]).astype(bf)        # [D, H]
        w1blk = np.ascontiguousarray(
            W1n.reshape(NH, 128, ND, 128).transpose(0, 3, 2, 1)
            .reshape(NH, 128, ND * 128))                          # [hc,d_in,(dc h_in)]
        w2blk = np.ascontiguousarray(
            W2n.T.reshape(NH, 128, D))                            # [hc,h_in,d]

        # ---- position ordering: (chunk, dst, slot) nested ----
        xglist = np.full(C, -1, np.int64)
        sctlist = np.full(C, 0, np.int64)
        p = 0
        for c in range(NC):
            for d in range(NCORES):
                toks = plan.tokens_ed[e][d]
                j0, j1 = c * PADC, min((c + 1) * PADC, len(toks))
                for j in range(j0, j1):
                    xglist[p] = toks[j]
                    sctlist[p] = c * E * PADC + d * PADC + (j - j0)
                    p += 1
        assert p == int(plan.cnt_ed[e].sum())
        for q in range(p, C):           # trailing pad slots:
            sctlist[q] = plan.R + (q % 128)   # scatter into a2a dump rows
        # indices are relative to each group's sliced out AP (a2a_in[goff:]),
        # which keeps later scatters disjoint from in-flight A2A chunk reads
        for goff, S in plan.groups:
            sctlist[goff:goff + S] -= goff
        assert (sctlist >= 0).all()
        # precompacted transposed x: per group g, free block
        # [ND*goff, ND*(goff+S)) holds [dc, tok] with d_in on partitions
        xc = np.zeros((C, D), dtype=bf)
        xc[:p] = xbf[xglist[:p]]
        parts = []
        for goff, S in plan.groups:
            blk = xc[goff:goff + S].reshape(S, ND, 128).transpose(2, 1, 0)
            parts.append(blk.reshape(128, ND * S))
        xtb = np.ascontiguousarray(np.concatenate(parts, axis=1))

        # ---- receive lists for core e's own token slice ----
        rcva = np.zeros(TSL, np.int64)
        rcvb = np.zeros(TSL, np.int64)
        jrank = {}
        for src in range(E):
            toks = plan.tokens_ed[src][e]
            for j, t in enumerate(toks):
                row = (j // PADC) * E * PADC + src * PADC + (j % PADC)
                jrank[(src, t)] = row
        for i in range(TSL):
            t = e * TSL + i
            e1, e2 = sorted(plan.top2[t])
            rcva[i] = jrank[(e1, t)]
            rcvb[i] = jrank[(e2, t)]

        lists = np.concatenate([
            _wrap16(sctlist), _wrap16(rcva), _wrap16(rcvb)], axis=1)

        im = {
            "xt": xtb,
            "w1d": w1blk,
            "w2d": w2blk,
            "lists": lists,
        }
        if plan.use_b1:
            im["b1w"] = np.ascontiguousarray(b1[e].reshape(NH, 128).T)
        if plan.use_b2:
            im["b2r"] = b2[e].reshape(1, D).astype(bf)
            im["ones1b"] = np.ones((1, 128), dtype=bf)
        in_maps.append(im)
    return in_maps


_COMPILED = {}


def get_compiled(plan: Plan):
    key = plan.key()
    if key not in _COMPILED:
        nc = bacc.Bacc("TRN2", target_bir_lowering=False, debug=False,
                       num_devices=NCORES)
        build_moe(nc, plan)
        nc.compile()
        _COMPILED[key] = nc
    return _COMPILED[key]


def unpermute(plan: Plan, shards):
    """Core d's out shard is token slice d in natural order."""
    return np.asarray(shards, np.float32).reshape(T, D)


def kernel(x, gate_v, gate_g, gate_b, w1_v, w1_g, b1, w2_v, w2_g, b2):
    from concourse.bass_utils import run_bass_kernel_spmd

    plan = make_plan(x, gate_v, gate_g, gate_b, b1, b2)
    nc = get_compiled(plan)
    in_maps = make_in_maps(plan, x, gate_v, gate_g, gate_b, w1_v, w1_g, b1,
                           w2_v, w2_g, b2)
    res = run_bass_kernel_spmd(nc, in_maps, core_ids=list(range(NCORES)))
    shards = [res.results[i]["out"] for i in range(NCORES)]
    out = unpermute(plan, np.stack(shards, axis=0))
    B, S_, D_ = x.shape
    return out.reshape(B, S_, D_)


# revision 22
# speedup vs baseline: 1.4878x; 1.0050x over previous
"""Expert-parallel MoE kernel for one TRN2 chip (8 NeuronCores) — v2.

All routing/bookkeeping is host-side; the device runs a pure FFN plus a
destination-grouped AllToAll:

  - Host computes gating in f64 (verified to reproduce the f32 reference
    top-2 exactly: min 2nd/3rd-logit margin is 2e-5, ~10x above f32 matmul
    rounding), builds per-core gather/scatter lists, normalizes + transposes
    + bf16-casts the weights, and blocks them for contiguous h-major DMA.
  - Core e owns expert e. Device: stream w1/w2 blocks in, gather compacted
    tokens (transposed) per 512-token group, GEMM1 (+Silu, bias folded),
    GEMM2 (+bias), scatter-add bf16 outputs into a destination-grouped
    AllToAll buffer, fire chunked AllToAlls that overlap later groups.
  - Receive: each core gathers the two expert contributions for each of its
    1024 tokens from the A2A output, adds them in fp32, DMAs the result out.

Compared to v1 this removes the on-device gating -> A2A -> prefix-sum ->
one-hot list build critical-path head, the 17MB ReduceScatter (A2A wire is
~5MB vs ~17MB), the rs_in zeroing, and the final cast pass.
"""

import numpy as np

import concourse.bass as bass
import concourse.mybir as mybir
import concourse.tile as tile
from concourse import bacc
from concourse.library_config import mlp

F32 = mybir.dt.float32
BF16 = mybir.dt.bfloat16
I16 = mybir.dt.int16

AX = mybir.AxisListType
OP = mybir.AluOpType
ACT = mybir.ActivationFunctionType

T, D, H, E = 8192, 1024, 4096, 8
NCORES = 8
TSL = T // NCORES
ND = D // 128
NH = H // 128


class Plan:
    """Host-computed routing plan; device program is keyed on `key()`."""

    def __init__(self, C, NC, PADC, use_b1, use_b2, gsize=512,
                 no_recv=False, no_coll=False, no_scatter=False, no_ffn=False):
        self.no_recv = no_recv
        self.no_coll = no_coll
        self.no_scatter = no_scatter
        self.no_ffn = no_ffn
        self.C = C                    # per-expert compacted capacity
        self.NC = NC                  # number of A2A chunks
        self.PADC = PADC              # slots per (expert,dst) pair per chunk
        self.use_b1 = use_b1
        self.use_b2 = use_b2
        self.groups = []
        off = 0
        while off < C:
            s = min(gsize, C - off)
            self.groups.append((off, s))
            off += s
        assert all(s % 128 == 0 for _, s in self.groups)
        self.R = NC * E * PADC        # a2a buffer rows
        assert self.R % 128 == 0
        # A2A chunk c may only fire once every position < B_c has been
        # scattered. P_{c+1} = sum_d min((c+1)*PADC, cnt_ed) <= 8*PADC*(c+1)
        # for every core, so firing after the first group whose end covers
        # that bound is uniform across cores (program must be SPMD).
        self.fire_after = {gi: [] for gi in range(len(self.groups))}
        for c in range(NC):
            bound = min(C, E * PADC * (c + 1))
            for gi, (goff, s) in enumerate(self.groups):
                if goff + s >= bound:
                    self.fire_after[gi].append(c)
                    break

    def key(self):
        return (self.C, self.NC, self.PADC, self.use_b1, self.use_b2,
                tuple(self.groups), self.no_recv, self.no_coll,
                self.no_scatter, self.no_ffn)


def build_moe(nc, plan: Plan):
    C, NC, PADC, R = plan.C, plan.NC, plan.PADC, plan.R
    B8 = E * PADC                     # rows per a2a chunk
    NLI = C // 16 + 2 * (TSL // 16)

    # ---------------- kernel I/O ----------------
    xt = nc.dram_tensor("xt", [128, C * ND], BF16,
                        kind="ExternalInput").ap()
    w1d = nc.dram_tensor("w1d", [NH, 128, ND * 128], BF16,
                         kind="ExternalInput").ap()
    w2d = nc.dram_tensor("w2d", [NH, 128, D], BF16, kind="ExternalInput").ap()
    lists = nc.dram_tensor("lists", [128, NLI], I16, kind="ExternalInput").ap()
    if plan.use_b1:
        b1w = nc.dram_tensor("b1w", [128, NH], F32, kind="ExternalInput").ap()
    if plan.use_b2:
        b2r = nc.dram_tensor("b2r", [1, D], BF16, kind="ExternalInput").ap()
        ones1b = nc.dram_tensor("ones1b", [1, 128], BF16,
                                kind="ExternalInput").ap()
    out_ext = nc.dram_tensor("out", [TSL, D], F32, kind="ExternalOutput").ap()

    # ---------------- internal DRAM ----------------
    # +128 dump rows: unfilled slots scatter there (negative scatter
    # indices wedge the device); the A2A only moves rows [0, R).
    a2a_in = nc.dram_tensor("a2a_in", [R + 128, D], BF16).ap()
    a2a_out = nc.dram_tensor("a2a_out", [R, D], BF16).ap()

    rg = [list(range(NCORES))]

    sct_off = 0
    rcva_off = C // 16
    rcvb_off = rcva_off + TSL // 16

    with tile.TileContext(nc) as tc:
        nc.gpsimd.load_library(mlp)

        with (
            tc.tile_pool(name="consts", bufs=1) as cpool,
            tc.tile_pool(name="wbig", bufs=1) as wpool,
        ):
            li = cpool.tile([128, NLI], I16)
            nc.sync.dma_start(li[:], lists)
            if plan.use_b1:
                b1w_sb = cpool.tile([128, NH], F32)
                nc.sync.dma_start(b1w_sb[:], b1w)
            if plan.use_b2:
                b2r_sb = cpool.tile([1, D], BF16)
                nc.sync.dma_start(b2r_sb[:], b2r)
                ones1b_sb = cpool.tile([1, 128], BF16)
                nc.sync.dma_start(ones1b_sb[:], ones1b)

            w1T = wpool.tile([128, NH, ND, 128], BF16)  # [d_in, hc, dc, h_in]
            w2T = wpool.tile([128, NH, D], BF16)        # [h_in, hc, d]
            for hc in range(NH):
                nc.scalar.dma_start(
                    w1T[:, hc, :, :].rearrange("p a b -> p (a b)"), w1d[hc])
            for hc in range(NH):
                nc.scalar.dma_start(w2T[:, hc, :], w2d[hc])

            # zero the a2a input buffer (scatter-add accumulates into it);
            # separate queue so weight DMAs keep priority on scalar's queue
            zt = cpool.tile([128, D], BF16)
            nc.gpsimd.memset(zt[:], 0.0)
            for j in range(R // 128):
                nc.scalar.dma_start(a2a_in[j * 128:(j + 1) * 128, :], zt[:])

            # ============ expert FFN over compacted tokens ============
            with (
                tc.tile_pool(name="gemm", bufs=2) as gpool,
                tc.tile_pool(name="psG", bufs=3, space="PSUM") as psp,
            ):
                for gi, (goff, S) in enumerate(plan.groups):
                    xgt = gpool.tile([128, ND, S], BF16, tag="xgt", bufs=2)
                    nc.sync.dma_start(
                        xgt[:].rearrange("p a b -> p (a b)"),
                        xt[:, ND * goff:ND * (goff + S)])
                    og = gpool.tile([128, S // 128, D], BF16, tag="og", bufs=2)
                    if plan.no_ffn:
                        nc.gpsimd.memset(og[:], 0.0)
                    if not plan.no_ffn:
                        hT = gpool.tile([128, NH, S], BF16, tag="hT", bufs=1)
                        for hc in range(NH):
                            ph = psp.tile([128, S], F32, tag="ph", bufs=3)
                            for dc in range(ND):
                                nc.tensor.matmul(
                                    ph[:], lhsT=w1T[:, hc, dc, :],
                                    rhs=xgt[:, dc, :],
                                    start=(dc == 0), stop=(dc == ND - 1))
                            if plan.use_b1:
                                nc.scalar.activation(hT[:, hc, :], ph[:],
                                                     ACT.Silu,
                                                     bias=b1w_sb[:, hc:hc + 1])
                            else:
                                nc.scalar.activation(hT[:, hc, :], ph[:],
                                                     ACT.Silu)
                        for tb in range(S // 128):
                            for dn in range(D // 512):
                                po = psp.tile([128, 512], F32, tag="po", bufs=3)
                                for hc in range(NH):
                                    nc.tensor.matmul(
                                        po[:],
                                        lhsT=hT[:, hc, tb * 128:(tb + 1) * 128],
                                        rhs=w2T[:, hc, dn * 512:(dn + 1) * 512],
                                        start=(hc == 0),
                                        stop=(hc == NH - 1 and not plan.use_b2))
                                if plan.use_b2:
                                    nc.tensor.matmul(
                                        po[:], lhsT=ones1b_sb[:],
                                        rhs=b2r_sb[:, dn * 512:(dn + 1) * 512],
                                        start=False, stop=True)
                                nc.vector.tensor_copy(
                                    og[:, tb, dn * 512:(dn + 1) * 512], po[:])
                    if plan.no_scatter:
                        nc.gpsimd.dma_start(
                            a2a_in[goff:goff + S, :].rearrange(
                                "(a p) b -> p a b", p=128),
                            og[:])
                    else:
                        nc.gpsimd.dma_scatter_add(
                            a2a_in[goff:, :], og[:],
                            li[:, sct_off + goff // 16:
                               sct_off + (goff + S) // 16],
                            S, S, D)
                    if not plan.no_coll:
                        for c in plan.fire_after[gi]:
                            nc.gpsimd.collective_compute(
                                "AllToAll", OP.bypass, replica_groups=rg,
                                ins=[a2a_in[c * B8:(c + 1) * B8, :]],
                                outs=[a2a_out[c * B8:(c + 1) * B8, :]])

            # ============ receive: out[t] = contrib_A[t] + contrib_B[t] ====
            with tc.tile_pool(name="outp", bufs=1) as opool:
                if plan.no_recv:
                    for c in range(TSL // 128):
                        rb = opool.tile([128, D], BF16, tag="rb", bufs=2)
                        nc.sync.dma_start(rb[:],
                                          a2a_out[c * 128:(c + 1) * 128, :])
                        rs = opool.tile([128, D], F32, tag="rs", bufs=4)
                        nc.vector.tensor_copy(rs[:], rb[:])
                        nc.sync.dma_start(out_ext[c * 128:(c + 1) * 128, :],
                                          rs[:])
                else:
                    ga = opool.tile([128, TSL // 128, D], BF16, tag="ga")
                    gb = opool.tile([128, TSL // 128, D], BF16, tag="gb")
                    nc.gpsimd.dma_gather(
                        ga[:], a2a_out, li[:, rcva_off:rcva_off + TSL // 16],
                        TSL, TSL, D, transpose=False)
                    nc.gpsimd.dma_gather(
                        gb[:], a2a_out, li[:, rcvb_off:rcvb_off + TSL // 16],
                        TSL, TSL, D, transpose=False)
                    for c in range(TSL // 128):
                        rs = opool.tile([128, D], F32, tag="rs", bufs=4)
                        nc.vector.tensor_tensor(rs[:], ga[:, c, :],
                                                gb[:, c, :], op=OP.add)
                        nc.sync.dma_start(out_ext[c * 128:(c + 1) * 128, :],
                                          rs[:])

    return nc


# ================= host-side planning =================

def _routing(x, gate_v, gate_g, gate_b):
    """Exact (f64) top-2 routing; matches the f32 jax reference."""
    xf = np.asarray(x, np.float64).reshape(T, D)
    gv = np.asarray(gate_v, np.float64)
    n = np.maximum(np.sqrt((gv * gv).sum(-1, keepdims=True)), 1e-12)
    gw = gv / n * np.asarray(gate_g, np.float64).reshape(E, 1)
    logits = xf @ gw.T + np.asarray(gate_b, np.float64).reshape(1, E)
    part = np.argpartition(-logits, 2, axis=1)[:, :2]
    return part  # [T, 2] unordered top-2 expert ids


def make_plan(x, gate_v, gate_g, gate_b, b1, b2, NC=6):
    top2 = _routing(x, gate_v, gate_g, gate_b)
    # tokens_ed[e][d]: sorted token ids of slice d routed to expert e
    tokens_ed = [[None] * NCORES for _ in range(E)]
    onehot = np.zeros((T, E), bool)
    onehot[np.arange(T), top2[:, 0]] = True
    onehot[np.arange(T), top2[:, 1]] = True
    for e in range(E):
        toks = np.nonzero(onehot[:, e])[0]
        for d in range(NCORES):
            tokens_ed[e][d] = toks[(toks >= d * TSL) & (toks < (d + 1) * TSL)]
    cnt_ed = np.array([[len(tokens_ed[e][d]) for d in range(NCORES)]
                       for e in range(E)])
    cnt_e = cnt_ed.sum(1)
    C = int(-(-cnt_e.max() // 128) * 128)
    max_ed = int(cnt_ed.max())
    PADC = int(-(-max_ed // NC // 16) * 16)
    use_b1 = bool(np.any(np.asarray(b1) != 0))
    use_b2 = bool(np.any(np.asarray(b2) != 0))
    plan = Plan(C, NC, PADC, use_b1, use_b2)
    plan.tokens_ed = tokens_ed
    plan.cnt_ed = cnt_ed
    plan.top2 = top2
    return plan


def _wrap16(v):
    """[n] int array -> [128, n/16] int16 list layout (entry i at
    [i % 16, i // 16], replicated across the 8 gpsimd channels)."""
    v = np.asarray(v)
    assert v.size % 16 == 0
    w = v.reshape(-1, 16).T.astype(np.int16)  # [16, n/16]
    return np.ascontiguousarray(np.tile(w, (8, 1)))


def make_in_maps(plan: Plan, x, gate_v, gate_g, gate_b, w1_v, w1_g, b1,
                 w2_v, w2_g, b2):
    import ml_dtypes

    bf = ml_dtypes.bfloat16
    C, NC, PADC = plan.C, plan.NC, plan.PADC
    f32 = np.float32

    xf = np.asarray(x, f32).reshape(T, D)
    xbf = xf.astype(bf)

    w1_v = np.asarray(w1_v, f32)
    w1_g = np.asarray(w1_g, f32)
    w2_v = np.asarray(w2_v, f32)
    w2_g = np.asarray(w2_g, f32)
    b1 = np.asarray(b1, f32)
    b2 = np.asarray(b2, f32)

    in_maps = []
    for e in range(E):
        # ---- normalized weights, blocked for h-major DMA ----
        n1 = np.maximum(np.sqrt((w1_v[e] ** 2).sum(-1, keepdims=True)), 1e-12)
        W1n = (w1_v[e] / n1 * w1_g[e][:, None]).astype(bf)        # [H, D]
        n2 = np.maximum(np.sqrt((w2_v[e] ** 2).sum(-1, keepdims=True)), 1e-12)
        W2n = (w2_v[e] / n2 * w2_g[e][:, None]).astype(bf)        # [D, H]
        w1blk = np.ascontiguousarray(
            W1n.reshape(NH, 128, ND, 128).transpose(0, 3, 2, 1)
            .reshape(NH, 128, ND * 128))                          # [hc,d_in,(dc h_in)]
        w2blk = np.ascontiguousarray(
            W2n.T.reshape(NH, 128, D))                            # [hc,h_in,d]

        # ---- position ordering: (chunk, dst, slot) nested ----
        xglist = np.full(C, -1, np.int64)
        sctlist = np.full(C, 0, np.int64)
        p = 0
        for c in range(NC):
            for d in range(NCORES):
                toks = plan.tokens_ed[e][d]
                j0, j1 = c * PADC, min((c + 1) * PADC, len(toks))
                for j in range(j0, j1):
                    xglist[p] = toks[j]
                    sctlist[p] = c * E * PADC + d * PADC + (j - j0)
                    p += 1
        assert p == int(plan.cnt_ed[e].sum())
        for q in range(p, C):           # trailing pad slots:
            sctlist[q] = plan.R + (q % 128)   # scatter into a2a dump rows
        # indices are relative to each group's sliced out AP (a2a_in[goff:]),
        # which keeps later scatters disjoint from in-flight A2A chunk reads
        for goff, S in plan.groups:
            sctlist[goff:goff + S] -= goff
        assert (sctlist >= 0).all()
        # precompacted transposed x: per group g, free block
        # [ND*goff, ND*(goff+S)) holds [dc, tok] with d_in on partitions
        xc = np.zeros((C, D), dtype=bf)
        xc[:p] = xbf[xglist[:p]]
        parts = []
        for goff, S in plan.groups:
            blk = xc[goff:goff + S].reshape(S, ND, 128).transpose(2, 1, 0)
            parts.append(blk.reshape(128, ND * S))
        xtb = np.ascontiguousarray(np.concatenate(parts, axis=1))

        # ---- receive lists for core e's own token slice ----
        rcva = np.zeros(TSL, np.int64)
        rcvb = np.zeros(TSL, np.int64)
        jrank = {}
        for src in range(E):
            toks = plan.tokens_ed[src][e]
            for j, t in enumerate(toks):
                row = (j // PADC) * E * PADC + src * PADC + (j % PADC)
                jrank[(src, t)] = row
        for i in range(TSL):
            t = e * TSL + i
            e1, e2 = sorted(plan.top2[t])
            rcva[i] = jrank[(e1, t)]
            rcvb[i] = jrank[(e2, t)]

        lists = np.concatenate([
            _wrap16(sctlist), _wrap16(rcva), _wrap16(rcvb)], axis=1)

        im = {
            "xt": xtb,
            "w1d": w1blk,
            "w2d": w2blk,
            "lists": lists,
        }
        if plan.use_b1:
            im["b1w"] = np.ascontiguousarray(b1[e].reshape(NH, 128).T)
        if plan.use_b2:
            im["b2r"] = b2[e].reshape(1, D).astype(bf)
            im["ones1b"] = np.ones((1, 128), dtype=bf)
        in_maps.append(im)
    return in_maps


_COMPILED = {}


def get_compiled(plan: Plan):
    key = plan.key()
    if key not in _COMPILED:
        nc = bacc.Bacc("TRN2", target_bir_lowering=False, debug=False,
                       num_devices=NCORES)
        build_moe(nc, plan)
        nc.compile()
        _COMPILED[key] = nc
    return _COMPILED[key]


def unpermute(plan: Plan, shards):
    """Core d's out shard is token slice d in natural order."""
    return np.asarray(shards, np.float32).reshape(T, D)


def kernel(x, gate_v, gate_g, gate_b, w1_v, w1_g, b1, w2_v, w2_g, b2):
    from concourse.bass_utils import run_bass_kernel_spmd

    plan = make_plan(x, gate_v, gate_g, gate_b, b1, b2)
    nc = get_compiled(plan)
    in_maps = make_in_maps(plan, x, gate_v, gate_g, gate_b, w1_v, w1_g, b1,
                           w2_v, w2_g, b2)
    res = run_bass_kernel_spmd(nc, in_maps, core_ids=list(range(NCORES)))
    shards = [res.results[i]["out"] for i in range(NCORES)]
    out = unpermute(plan, np.stack(shards, axis=0))
    B, S_, D_ = x.shape
    return out.reshape(B, S_, D_)
